# revision 36
# baseline (speedup 1.0000x reference)
"""Causal self-attention (B=2, T=2048, D=1024, H=16) on 8 TRN2 NeuronCores.

Sharding: data-parallel over batch (2) x tensor-parallel over head groups (4).
Each core handles 1 batch x 4 heads: Wq/Wk/Wv column-sharded, Wo row-sharded;
each core emits a partial (T, D) output (bf16) and the host sums 4 partials
per batch in fp32.

Numerics/layout (all matmul inputs low-precision, fp32 PSUM accumulation):
  - x is pre-transposed on the HOST and shipped as x^T in fp8e4m3 (slab-major
    so each 512-token slab is one contiguous DMA). No PE transposes at all.
  - Q/K/V projections run in fp8 DoubleRow mode: lhsT [128, 2, 128] packs a
    pair of 128-row d-chunks of W, rhs [128, 2, 512] the matching x^T pair,
    contracting 256 rows per pass (2x PE throughput).
  - S = K^T.T @ Q^T in bf16 with the zero-padded parity layout (kTz0/kTz1)
    so the stationary stream stays 128 rows for the PE clock gate. Causal
    masking via bmask matmul adds (identity lhsT, bf16).
  - exp on ACT writes A directly as fp8e4m3; A.V runs in fp8 DoubleRow with
    lhsT = vp k-tile pairs [128, 2, 128] and rhs = the exp output viewed as
    [128, 2, 512] (pair layout falls out of the S pair tiles for free).
    V' keeps the parity padding (even h: [v|one|0], odd h: [one|0|v]) so the
    denominator accumulates in a spare opsum row during A.V itself.
  - denominators: DVE copies the denom row out of PSUM, then a single DVE
    reciprocal writes 1/D (bf16) straight into row h of a zeroed [128, 512]
    rhs tile; one full-K=128 matmul per (qs, head-pair) broadcasts both
    heads' 1/D across partitions (no rank-1 matmuls -> no HAM clock dips).
  - o^T normalized on DVE/gpsimd, Wo projection + y in bf16.

Schedule: one fused pipeline over q-slabs; step i: S(i), exp(i-1), AV(i-2).
Projections of slab ts+1 and V-tile evictions are spread as deferred
closures; ytiles are deferred INTO later slabs (qs=0 -> slab 1, qs=1,2 ->
slab 3) because slab 3 has no projection work and the ACT engine otherwise
outruns the PE there. Preamble DMAs split across sync/scalar/gpsimd queues.
"""

import sys, os, types

sys.path.insert(0, "/opt/trn_rl_repo")

import numpy as np
import ml_dtypes
from contextlib import ExitStack

import concourse.bass as bass
import concourse.mybir as mybir
import concourse.tile as tile
from concourse import bacc
from concourse.masks import make_identity

B, T, D, H = 2, 2048, 1024, 16
DH = D // H          # 64
NCORES = 8
HG = 4               # heads per core
F = HG * DH          # 256 local features per core
P = 128
F32 = mybir.dt.float32
BF16 = mybir.dt.bfloat16
F8 = mybir.dt.float8e4
NEG = -1.0e9
DR = mybir.MatmulPerfMode.DoubleRow

TT = T // P          # 16 k-tiles
QS = T // 512        # 4 q-slabs
DC = D // P          # 8 d-chunks
NPR = DC // 2        # 4 d-chunk pairs

LAST_RESULTS = None  # BassKernelResults of the most recent hardware run


def _install_ntff_hook():
    if "antenv.axon_hooks" in sys.modules:
        return
    try:
        import antenv
        from trn_agent_boot.trn_boot import _ntff_profile_via_ctypes

        m = types.ModuleType("antenv.axon_hooks")
        h = _ntff_profile_via_ctypes("/opt/axon/libaxon_pjrt.so")
        m.get_axon_ntff_profile_hook = lambda: h
        m.set_axon_ntff_profile_hook = lambda hh: None
        sys.modules["antenv.axon_hooks"] = m
        antenv.axon_hooks = m
    except Exception:
        pass


def build_nc():
    nc = bacc.Bacc("TRN2", target_bir_lowering=False, debug=False)

    # x^T, slab-major: [p, ts, dc, j] = x[ts*512+j, dc*128+p], bf16
    xt_d = nc.dram_tensor("xt", [P, QS * DC * 512], BF16, kind="ExternalInput").ap()
    # W chunk-major: [p, dc, f] = W[dc*128+p, f], bf16
    wq_d = nc.dram_tensor("wq", [P, DC * F], BF16, kind="ExternalInput").ap()
    wk_d = nc.dram_tensor("wk", [P, DC * F], BF16, kind="ExternalInput").ap()
    wv_d = nc.dram_tensor("wv", [P, DC * F], BF16, kind="ExternalInput").ap()
    # Wo partition-major bf16: [p, fc, dout] = Wo[fc*128+p, dout]
    wo_d = nc.dram_tensor("wo", [P, 2 * D], BF16, kind="ExternalInput").ap()
    bm_d = nc.dram_tensor("bm", [P, 512], BF16, kind="ExternalInput").ap()
    z_d = nc.dram_tensor("z", [P, 4096], BF16, kind="ExternalInput").ap()
    vpi_d = nc.dram_tensor("vpi", [P, TT * HG * P], BF16, kind="ExternalInput").ap()
    y_d = nc.dram_tensor("y", [T, D], BF16, kind="ExternalOutput").ap()

    with tile.TileContext(nc) as tc, ExitStack() as ctx:
        ctx.enter_context(nc.allow_low_precision("intentional bf16/fp8 kernel"))
        const = ctx.enter_context(tc.tile_pool(name="const", bufs=1))
        wpool = ctx.enter_context(tc.tile_pool(name="wpool", bufs=1))
        qkv = ctx.enter_context(tc.tile_pool(name="qkv", bufs=1))
        sp_ps = ctx.enter_context(tc.tile_pool(name="sp_ps", bufs=3, space="PSUM"))
        o_ps = ctx.enter_context(tc.tile_pool(name="o_ps", bufs=2, space="PSUM"))
        ptp = ctx.enter_context(tc.tile_pool(name="ptp", bufs=4))
        stg = ctx.enter_context(tc.tile_pool(name="stg", bufs=8))
        ysb = ctx.enter_context(tc.tile_pool(name="ysb", bufs=3))

        # ---- constants ----
        ident = const.tile([P, P], F32, name="ident")
        make_identity(nc, ident)
        identb = const.tile([P, P], BF16, name="identb")
        nc.scalar.copy(identb[:], ident[:])
        bmask = const.tile([P, 512], BF16, name="bmask")
        # osel[:, h, :]: full-K=128 stationary that broadcasts zrhs row 32h
        # onto the 64 output partitions of head h's parity (rank-1 values,
        # K=128 occupancy so the PE clock gate sees a full stream; row 32h
        # because engine writes must start at a 32-aligned partition)
        # oselP: one stationary for both parities -- staging row 64 (even
        # head's D) broadcasts to output rows 0:64, staging row 0 (odd
        # head's D) to rows 64:128
        oselP = const.tile([P, P], BF16, name="oselP")
        nc.vector.memset(oselP[:], 0.0)
        nc.vector.memset(oselP[64:65, 0:64], 1.0)
        nc.vector.memset(oselP[0:1, 64:128], 1.0)
        # per-head-pair persistent denominator staging rows (zeros elsewhere
        # so the PE broadcast never sees NaN garbage in unused partitions)
        dstages = []
        for jb in range(2):
            t = const.tile([P, 512], BF16, name=f"dstage{jb}")
            nc.vector.memset(t[:], 0.0)
            dstages.append(t)
        # PE warmup: the DMA rings take ~8us to start moving data at kernel
        # begin; dummy matmuls keep the PE pstate ramping so the first real
        # projections run at 2.4GHz instead of cold-clock
        junk = const.tile([P, 512], BF16, name="junk")
        nc.vector.memset(junk[:], 0.0)
        for _ in range(16):
            jp = sp_ps.tile([P, 512], F32, name="jp", tag="sp")
            nc.tensor.matmul(out=jp[:], lhsT=identb[:], rhs=junk[:], start=True, stop=True)
        # softmax shift (cancels in normalization; keeps exp under fp8 max)
        nbias = const.tile([P, 1], F32, name="nbias")
        nc.vector.memset(nbias[:], -2.5)
        # touch Exp early so the ACT table load happens in the idle preamble
        warm = const.tile([1, 1], F32, name="warm")
        warm_in = const.tile([1, 1], F32, name="warm_in")
        nc.vector.memset(warm_in[:], 0.0)
        nc.scalar.activation(warm[:], warm_in[:], mybir.ActivationFunctionType.Exp)

        # ---- persistent tensors ----
        wq_s = wpool.tile([P, DC, F], BF16, name="wq_s")
        wk_s = wpool.tile([P, DC, F], BF16, name="wk_s")
        wv_s = wpool.tile([P, DC, F], BF16, name="wv_s")
        wo2 = wpool.tile([P, 2, D], BF16, name="wo2")
        xT = qkv.tile([P, QS, DC, 512], BF16, name="xT")
        qT = qkv.tile([P, 2, T], BF16, name="qT")        # [2 heads x dh, jb, t]
        kTz0 = qkv.tile([P, 2, T], BF16, name="kTz0")    # [k_even; 0]
        kTz1 = qkv.tile([P, 2, T], BF16, name="kTz1")    # [0; k_odd]
        vp = qkv.tile([P, TT, HG, P], BF16, name="vp")     # padded V', parity layouts
        oT = qkv.tile([P, 2, T], BF16, name="oT")        # normalized o^T [f, t]

        # ---- preamble ----
        # x slab 0 first on sync (gates the first projections)
        nc.sync.dma_start(out=xT[:, 0], in_=xt_d[:, 0:DC * 512])
        # bmask is needed by the very first S step (qs=0 is all-diagonal)
        nc.scalar.dma_start(out=bmask[:], in_=bm_d[:])
        nc.scalar.dma_start(out=wq_s[:], in_=wq_d[:])
        nc.scalar.dma_start(out=wk_s[:], in_=wk_d[:])
        # zero/ones fills arrive as DMA images (kz once; vp slab 0 now,
        # rest per slab)
        nc.sync.dma_start(out=kTz0[64:128, :, :], in_=z_d[64:128, :])
        nc.sync.dma_start(out=kTz1[0:64, :, :], in_=z_d[0:64, :])
        nc.sync.dma_start(out=vp[:, 0:4], in_=vpi_d[:, 0:4 * HG * P])

        # ---- emission helpers ----
        def _act_recip(out, in_):
            eng = nc.scalar
            inputs = [eng.lower_ap(in_)]
            for arg in (0.0, 1.0, 0.0):
                inputs.append(mybir.ImmediateValue(dtype=mybir.dt.float32, value=arg))
            return eng.add_instruction(mybir.InstActivation(
                name=nc.get_next_instruction_name(),
                func=mybir.ActivationFunctionType.Reciprocal,
                ins=inputs, outs=[eng.lower_ap(out)]))

        ei = 0

        def emit_slab(ts):
            nonlocal ei
            if ts + 1 < QS:
                nc.sync.dma_start(out=xT[:, ts + 1],
                                  in_=xt_d[:, (ts + 1) * DC * 512:(ts + 2) * DC * 512])
                t1 = ts + 1
                nc.sync.dma_start(out=vp[:, 4 * t1:4 * t1 + 4],
                                  in_=vpi_d[:, 4 * t1 * HG * P:(4 * t1 + 4) * HG * P])
            if ts == 0:
                nc.scalar.dma_start(out=wv_s[:], in_=wv_d[:])
                nc.scalar.dma_start(out=wo2[:], in_=wo_d[:])
            sl = slice(ts * 512, (ts + 1) * 512)

            def emit_proj(w_s, which, jb):
                nonlocal ei
                pp = sp_ps.tile([P, 512], F32, name="pp", tag="sp")
                for dc in range(DC):
                    nc.tensor.matmul(
                        out=pp[:],
                        lhsT=w_s[:, dc, jb * P:(jb + 1) * P],
                        rhs=xT[:, ts, dc, :],
                        start=(dc == 0),
                        stop=(dc == DC - 1),
                    )
                if which == "q":
                    nc.vector.tensor_copy(qT[:, jb, sl], pp[:])
                else:
                    nc.vector.tensor_copy(kTz0[0:64, jb, sl], pp[0:64, :])
                    nc.vector.tensor_copy(kTz1[64:128, jb, sl], pp[64:128, :])
                ei += 1

            deferred = []
            if ts == 0:
                emit_proj(wq_s, "q", 0)
                emit_proj(wk_s, "k", 0)
                emit_proj(wq_s, "q", 1)
                emit_proj(wk_s, "k", 1)
            else:
                for jb in range(2):
                    emit_proj(wq_s, "q", jb)
                for jb in range(2):
                    deferred.append(lambda b=jb: emit_proj(wk_s, "k", b))

            def emit_v(j, tt):
                nonlocal ei
                pv = sp_ps.tile([P, F], F32, name="pv", tag="sp")
                for dc in range(DC):
                    nc.tensor.matmul(
                        out=pv[:],
                        lhsT=xT[:, ts, dc, j * P:(j + 1) * P],
                        rhs=wv_s[:, dc, :],
                        start=(dc == 0),
                        stop=(dc == DC - 1),
                    )
                pvv = pv[:].rearrange("p (hp par dh) -> p hp par dh", hp=2, par=2, dh=DH)
                ve = vp[:, tt, :, :].rearrange("p (hp par) c -> p hp par c", par=2)
                nc.vector.tensor_copy(ve[:, :, 0, 0:DH], pvv[:, :, 0, :])
                nc.vector.tensor_copy(ve[:, :, 1, DH:P], pvv[:, :, 1, :])
                ei += 1
            return deferred + [(lambda a=j_, b=tt_: emit_v(a, b)) for j_, tt_ in enumerate(range(4 * ts, 4 * ts + 4))]

        state = {}

        def emit_S(qs, h, kp):
            jbh, par = h // 2, h % 2
            kTz = kTz0 if par == 0 else kTz1
            q0 = qs * 512
            spair = sp_ps.tile([P, 1024], F32, name="spair", tag="sp")
            for half in range(2):
                kt = 2 * kp + half
                k0 = kt * P
                sreg = spair[:, half * 512:(half + 1) * 512]
                lhsk = kTz[:, jbh, k0:k0 + P]
                rhsq = qT[:, jbh, :]
                if k0 >= q0:
                    d = k0 - q0
                    nc.tensor.matmul(out=sreg[:, 0:d + P], lhsT=identb[:],
                                     rhs=bmask[:, 384 - d:512], start=True, stop=False)
                    nc.tensor.matmul(out=sreg[:, d:d + P], lhsT=lhsk,
                                     rhs=rhsq[:, q0 + d:q0 + d + P],
                                     start=False, stop=(d == 384))
                    if d < 384:
                        nc.tensor.matmul(out=sreg[:, d + P:512], lhsT=lhsk,
                                         rhs=rhsq[:, q0 + d + P:q0 + 512],
                                         start=False, stop=True)
                else:
                    nc.tensor.matmul(out=sreg, lhsT=lhsk,
                                     rhs=rhsq[:, q0:q0 + 512],
                                     start=True, stop=True)
            state[(qs, h, kp)] = spair

        def emit_exp(qs, h, kp):
            spair = state[(qs, h, kp)]
            q0 = qs * 512
            d0 = max(2 * kp * P - q0, 0)
            # bias shifts the softmax uniformly (cancels in normalization);
            # keeps exp(S) well under fp8e4m3's max finite value
            pt = ptp.tile([P, 1024], BF16, name="pt")
            nc.scalar.activation(pt[:, d0:1024], spair[:, d0:1024],
                                 mybir.ActivationFunctionType.Exp,
                                 scale=0.125, bias=nbias[:])
            state[(qs, h, kp)] = (spair, pt)

        def emit_AV(qs, h, kp, npr_steps):
            _, pt = state.pop((qs, h, kp))
            q0 = qs * 512
            if kp == 0:
                state[(qs, h)] = o_ps.tile([P, 512], F32, name="opsum")
            opsum = state[(qs, h)]
            for half in range(2):
                kt = 2 * kp + half
                d = max(kt * P - q0, 0)
                nc.tensor.matmul(
                    out=opsum[:, d:512],
                    lhsT=vp[:, kt, h, :],
                    rhs=pt[:, half * 512 + d:half * 512 + 512],
                    start=(kt == 0),
                    stop=(kt == 2 * npr_steps - 1),
                )

        def emit_normA(qs, h):
            # stage the denominator row (lane-preserving copy into the
            # pair's persistent zero tile: row 64 even head, row 0 odd head)
            opsum = state[(qs, h)]
            r = DH if h % 2 == 0 else 0
            nc.vector.tensor_copy(dstages[h // 2][r:r + 1, :], opsum[r:r + 1, :])

        def emit_normE(qs, h):
            # evict o unnormalized; frees the PSUM accumulator early and
            # decouples the normalization chain from the opsum lifetime
            opsum = state.pop((qs, h))
            jbh, par = h // 2, h % 2
            rows = slice(par * 64, (par + 1) * 64)
            q0 = qs * 512
            nc.vector.tensor_copy(oT[rows, jbh, q0:q0 + 512], opsum[rows, :])

        def emit_normBa(qs, jbh):
            # PE broadcast of both heads' D + one exact DVE reciprocal for
            # the pair (cost scales with free size only)
            bcD = sp_ps.tile([P, 512], F32, name="bcD", tag="sp")
            nc.tensor.matmul(out=bcD[:], lhsT=oselP[:], rhs=dstages[jbh][:],
                             start=True, stop=True)
            rbt = stg.tile([P, 512], BF16, name="rbt")
            nc.vector.reciprocal(rbt[:], bcD[:])
            state[(qs, jbh, "rbt")] = rbt

        def emit_normF(qs, jbh):
            # in-place normalize both heads of the pair: all-SBUF bf16
            rbt = state.pop((qs, jbh, "rbt"))
            q0 = qs * 512
            sl_ = oT[:, jbh, q0:q0 + 512]
            nc.vector.tensor_mul(sl_, sl_, rbt[:])

        def emit_ytile(qs, tt, e):
            yp = sp_ps.tile([P, 1024], F32, name="yp", tag="sp")
            for jh in range(2):
                for fc in range(2):
                    nc.tensor.matmul(
                        out=yp[:, jh * 512:(jh + 1) * 512],
                        lhsT=oT[:, fc, tt * P:(tt + 1) * P],
                        rhs=wo2[:, fc, jh * 512:(jh + 1) * 512],
                        start=(fc == 0),
                        stop=(fc == 1),
                    )
            yt = ysb.tile([P, D], BF16, name="yt")
            nc.vector.tensor_copy(yt[:], yp[:])
            nc.sync.dma_start(out=y_d[tt * P:(tt + 1) * P, :], in_=yt[:])

        # ---- fused pipeline ----
        steps = []
        for qs in range(QS):
            for h in range(HG):
                nprs = 2 * qs + 2
                for kp in range(nprs):
                    steps.append((qs, h, kp, nprs))
        first_step_of_qs = {}
        step_index_of_qs = {}
        for i, (qs, h, kp, nprs) in enumerate(steps):
            if (h, kp) == (0, 0):
                first_step_of_qs[i] = qs
                step_index_of_qs[qs] = i

        # ytile deferral: qs=0 -> slab 1 (late), qs=1 -> slab 3 (early),
        # qs=2 -> slab 3 (mid), qs=3 -> drain. Keys are absolute step indices.
        ytile_keys = {}
        s1, s2, s3 = step_index_of_qs[1], step_index_of_qs[2], step_index_of_qs[3]
        for j in range(4):
            ytile_keys[(0, j)] = s1 + 8 + 2 * j
            ytile_keys[(1, j)] = s2 + 12 + 3 * j
            ytile_keys[(2, j)] = s3 + 6 + 4 * j
            ytile_keys[(3, j)] = 10 ** 9
        todo = []

        def flush(i):
            while todo and todo[0][0] <= i:
                todo.pop(0)[1]()

        def push(key, fn):
            todo.append((key, fn))
            todo.sort(key=lambda e: e[0])

        nsteps = len(steps)
        for i in range(nsteps):
            if i in first_step_of_qs:
                for vj, fn_ in enumerate(emit_slab(first_step_of_qs[i])):
                    push(i + vj, fn_)
            qs, h, kp, nprs = steps[i]
            emit_S(qs, h, kp)
            flush(i)
            if i >= 1:
                emit_exp(*steps[i - 1][:3])
            if i >= 3:
                pqs, ph_, pkp, pnprs = steps[i - 3]
                emit_AV(pqs, ph_, pkp, pnprs)
                if pkp == pnprs - 1:
                    emit_normA(pqs, ph_)
                    push(i + 2, lambda q=pqs, hh=ph_: emit_normE(q, hh))
                    if ph_ % 2 == 1:
                        ka = 3 if ph_ == 1 else 7
                        push(i + ka, lambda q=pqs, jb=ph_ // 2: emit_normBa(q, jb))
                        push(i + ka + 2, lambda q=pqs, jb=ph_ // 2: emit_normF(q, jb))
                    if ph_ == HG - 1:
                        for j, tt in enumerate(range(4 * pqs, 4 * pqs + 4)):
                            push(max(ytile_keys[(pqs, j)], i + 10 + j),
                                 lambda q=pqs, t_=tt, e=j: emit_ytile(q, t_, e))
        # drain
        emit_exp(*steps[nsteps - 1][:3])
        for i in (nsteps - 3, nsteps - 2, nsteps - 1):
            qs, h, kp, nprs = steps[i]
            emit_AV(qs, h, kp, nprs)
            if kp == nprs - 1:
                emit_normA(qs, h)
                push(10 ** 9 - 3, lambda q=qs, hh=h: emit_normE(q, hh))
                if h % 2 == 1:
                    push(10 ** 9 - 2, lambda q=qs, jb=h // 2: emit_normBa(q, jb))
                    push(10 ** 9 - 1, lambda q=qs, jb=h // 2: emit_normF(q, jb))
                if h == HG - 1:
                    for j, tt in enumerate(range(4 * qs, 4 * qs + 4)):
                        push(10 ** 9, lambda q=qs, t_=tt, e=j: emit_ytile(q, t_, e))
        for _, fn in todo:
            fn()

    nc.compile()
    return nc


def make_mask():
    # BM[k, j] = -1e9 if j < 384 + k else 0
    j = np.arange(512)[None, :]
    k = np.arange(P)[:, None]
    return np.where(j < 384 + k, np.float32(NEG), np.float32(0.0))


BF16NP = ml_dtypes.bfloat16


def make_core_inputs(x, Wq, Wk, Wv, Wo):
    bm = make_mask().astype(BF16NP)
    zeros = np.zeros((P, 4096), dtype=BF16NP)
    # initial V' image: zeros with the denominator ones-columns set
    vpi4 = np.zeros((P, TT, HG, P), dtype=np.float32)
    for h in range(HG):
        c = DH if h % 2 == 0 else 0
        vpi4[:, :, h, c] = 1.0
    vpi = np.ascontiguousarray(vpi4.reshape(P, -1)).astype(BF16NP)
    in_maps = []
    xtb = {}
    for b in range(B):
        # [p, ts, dc, j] = x[b][ts*512+j, dc*128+p]
        xt = np.ascontiguousarray(
            x[b].T.reshape(DC, P, QS, 512).transpose(1, 2, 0, 3).reshape(P, -1))
        xtb[b] = xt.astype(BF16NP)

    for c in range(NCORES):
        b, hg = c // HG, c % HG
        s = slice(hg * F, (hg + 1) * F)

        def wdr(w):  # [1024, F] -> [p, dc, f] bf16
            return np.ascontiguousarray(
                w.reshape(DC, P, F).transpose(1, 0, 2).reshape(P, -1)
            ).astype(BF16NP)

        wo = np.ascontiguousarray(
            Wo[s, :].reshape(2, P, D).transpose(1, 0, 2).reshape(P, -1)
        ).astype(BF16NP)

        in_maps.append({
            "xt": xtb[b],
            "wq": wdr(Wq[:, s]),
            "wk": wdr(Wk[:, s]),
            "wv": wdr(Wv[:, s]),
            "wo": wo,
            "bm": bm,
            "z": zeros,
            "vpi": vpi,
        })
    return in_maps


_NC_CACHE = None


def _get_nc():
    global _NC_CACHE
    if _NC_CACHE is None:
        _NC_CACHE = build_nc()
    return _NC_CACHE


def kernel(x, Wq, Wk, Wv, Wo):
    global LAST_RESULTS
    _install_ntff_hook()
    from concourse.bass_utils import run_bass_kernel_spmd

    x = np.asarray(x, dtype=np.float32)
    Wq = np.asarray(Wq, dtype=np.float32)
    Wk = np.asarray(Wk, dtype=np.float32)
    Wv = np.asarray(Wv, dtype=np.float32)
    Wo = np.asarray(Wo, dtype=np.float32)

    nc = _get_nc()
    in_maps = make_core_inputs(x, Wq, Wk, Wv, Wo)
    res = run_bass_kernel_spmd(nc, in_maps, list(range(NCORES)))
    LAST_RESULTS = res

    out = np.zeros((B, T, D), dtype=np.float32)
    for c in range(NCORES):
        out[c // HG] += res.results[c]["y"].astype(np.float32)
    return out


# revision 37
# speedup vs baseline: 1.0164x; 1.0164x over previous
"""Causal self-attention (B=2, T=2048, D=1024, H=16) on 8 TRN2 NeuronCores.

Sharding: data-parallel over batch (2) x tensor-parallel over head groups (4).
Each core handles 1 batch x 4 heads: Wq/Wk/Wv column-sharded, Wo row-sharded;
each core emits a partial (T, D) output (bf16) and the host sums 4 partials
per batch in fp32.

Numerics/layout (all matmul inputs low-precision, fp32 PSUM accumulation):
  - x is pre-transposed on the HOST and shipped as x^T in fp8e4m3 (slab-major
    so each 512-token slab is one contiguous DMA). No PE transposes at all.
  - Q/K/V projections run in fp8 DoubleRow mode: lhsT [128, 2, 128] packs a
    pair of 128-row d-chunks of W, rhs [128, 2, 512] the matching x^T pair,
    contracting 256 rows per pass (2x PE throughput).
  - S = K^T.T @ Q^T in bf16 with the zero-padded parity layout (kTz0/kTz1)
    so the stationary stream stays 128 rows for the PE clock gate. Causal
    masking via bmask matmul adds (identity lhsT, bf16).
  - exp on ACT writes A directly as fp8e4m3; A.V runs in fp8 DoubleRow with
    lhsT = vp k-tile pairs [128, 2, 128] and rhs = the exp output viewed as
    [128, 2, 512] (pair layout falls out of the S pair tiles for free).
    V' keeps the parity padding (even h: [v|one|0], odd h: [one|0|v]) so the
    denominator accumulates in a spare opsum row during A.V itself.
  - denominators: DVE copies the denom row out of PSUM, then a single DVE
    reciprocal writes 1/D (bf16) straight into row h of a zeroed [128, 512]
    rhs tile; one full-K=128 matmul per (qs, head-pair) broadcasts both
    heads' 1/D across partitions (no rank-1 matmuls -> no HAM clock dips).
  - o^T normalized on DVE/gpsimd, Wo projection + y in bf16.

Schedule: one fused pipeline over q-slabs; step i: S(i), exp(i-1), AV(i-2).
Projections of slab ts+1 and V-tile evictions are spread as deferred
closures; ytiles are deferred INTO later slabs (qs=0 -> slab 1, qs=1,2 ->
slab 3) because slab 3 has no projection work and the ACT engine otherwise
outruns the PE there. Preamble DMAs split across sync/scalar/gpsimd queues.
"""

import sys, os, types

sys.path.insert(0, "/opt/trn_rl_repo")

import numpy as np
import ml_dtypes
from contextlib import ExitStack

import concourse.bass as bass
import concourse.mybir as mybir
import concourse.tile as tile
from concourse import bacc
from concourse.masks import make_identity

B, T, D, H = 2, 2048, 1024, 16
DH = D // H          # 64
NCORES = 8
HG = 4               # heads per core
F = HG * DH          # 256 local features per core
P = 128
F32 = mybir.dt.float32
BF16 = mybir.dt.bfloat16
F8 = mybir.dt.float8e4
NEG = -1.0e9
DR = mybir.MatmulPerfMode.DoubleRow

TT = T // P          # 16 k-tiles
QS = T // 512        # 4 q-slabs
DC = D // P          # 8 d-chunks
NPR = DC // 2        # 4 d-chunk pairs

LAST_RESULTS = None  # BassKernelResults of the most recent hardware run


def _install_ntff_hook():
    if "antenv.axon_hooks" in sys.modules:
        return
    try:
        import antenv
        from trn_agent_boot.trn_boot import _ntff_profile_via_ctypes

        m = types.ModuleType("antenv.axon_hooks")
        h = _ntff_profile_via_ctypes("/opt/axon/libaxon_pjrt.so")
        m.get_axon_ntff_profile_hook = lambda: h
        m.set_axon_ntff_profile_hook = lambda hh: None
        sys.modules["antenv.axon_hooks"] = m
        antenv.axon_hooks = m
    except Exception:
        pass


def build_nc():
    nc = bacc.Bacc("TRN2", target_bir_lowering=False, debug=False)

    # x^T, slab-major: [p, ts, dc, j] = x[ts*512+j, dc*128+p], bf16
    xt_d = nc.dram_tensor("xt", [P, QS * DC * 512], BF16, kind="ExternalInput").ap()
    # W chunk-major: [p, dc, f] = W[dc*128+p, f], bf16
    wq_d = nc.dram_tensor("wq", [P, DC * F], BF16, kind="ExternalInput").ap()
    wk_d = nc.dram_tensor("wk", [P, DC * F], BF16, kind="ExternalInput").ap()
    wv_d = nc.dram_tensor("wv", [P, DC * F], BF16, kind="ExternalInput").ap()
    # Wo partition-major bf16: [p, fc, dout] = Wo[fc*128+p, dout]
    wo_d = nc.dram_tensor("wo", [P, 2 * D], BF16, kind="ExternalInput").ap()
    bm_d = nc.dram_tensor("bm", [P, 512], BF16, kind="ExternalInput").ap()
    z_d = nc.dram_tensor("z", [P, 4096], BF16, kind="ExternalInput").ap()
    vpi_d = nc.dram_tensor("vpi", [P, TT * HG * P], BF16, kind="ExternalInput").ap()
    y_d = nc.dram_tensor("y", [T, D], BF16, kind="ExternalOutput").ap()

    with tile.TileContext(nc) as tc, ExitStack() as ctx:
        ctx.enter_context(nc.allow_low_precision("intentional bf16/fp8 kernel"))
        const = ctx.enter_context(tc.tile_pool(name="const", bufs=1))
        wpool = ctx.enter_context(tc.tile_pool(name="wpool", bufs=1))
        qkv = ctx.enter_context(tc.tile_pool(name="qkv", bufs=1))
        sp_ps = ctx.enter_context(tc.tile_pool(name="sp_ps", bufs=3, space="PSUM"))
        o_ps = ctx.enter_context(tc.tile_pool(name="o_ps", bufs=2, space="PSUM"))
        ptp = ctx.enter_context(tc.tile_pool(name="ptp", bufs=4))
        stg = ctx.enter_context(tc.tile_pool(name="stg", bufs=8))
        ysb = ctx.enter_context(tc.tile_pool(name="ysb", bufs=3))

        # ---- constants ----
        ident = const.tile([P, P], F32, name="ident")
        make_identity(nc, ident)
        identb = const.tile([P, P], BF16, name="identb")
        nc.scalar.copy(identb[:], ident[:])
        bmask = const.tile([P, 512], BF16, name="bmask")
        # osel[:, h, :]: full-K=128 stationary that broadcasts zrhs row 32h
        # onto the 64 output partitions of head h's parity (rank-1 values,
        # K=128 occupancy so the PE clock gate sees a full stream; row 32h
        # because engine writes must start at a 32-aligned partition)
        # oselP: one stationary for both parities -- staging row 64 (even
        # head's D) broadcasts to output rows 0:64, staging row 0 (odd
        # head's D) to rows 64:128
        oselP = const.tile([P, P], BF16, name="oselP")
        nc.vector.memset(oselP[:], 0.0)
        nc.vector.memset(oselP[64:65, 0:64], 1.0)
        nc.vector.memset(oselP[0:1, 64:128], 1.0)
        # per-head-pair persistent denominator staging rows (zeros elsewhere
        # so the PE broadcast never sees NaN garbage in unused partitions)
        dstages = []
        for jb in range(2):
            t = const.tile([P, 512], BF16, name=f"dstage{jb}")
            nc.vector.memset(t[:], 0.0)
            dstages.append(t)
        # PE warmup: the DMA rings take ~8us to start moving data at kernel
        # begin; dummy matmuls keep the PE pstate ramping so the first real
        # projections run at 2.4GHz instead of cold-clock
        junk = const.tile([P, 512], BF16, name="junk")
        nc.vector.memset(junk[:], 0.0)
        for _ in range(16):
            jp = sp_ps.tile([P, 512], F32, name="jp", tag="sp")
            nc.tensor.matmul(out=jp[:], lhsT=identb[:], rhs=junk[:], start=True, stop=True)
        # softmax shift (cancels in normalization; keeps exp under fp8 max)
        nbias = const.tile([P, 1], F32, name="nbias")
        nc.vector.memset(nbias[:], -2.5)
        # touch Exp early so the ACT table load happens in the idle preamble
        warm = const.tile([1, 1], F32, name="warm")
        warm_in = const.tile([1, 1], F32, name="warm_in")
        nc.vector.memset(warm_in[:], 0.0)
        nc.scalar.activation(warm[:], warm_in[:], mybir.ActivationFunctionType.Exp)

        # ---- persistent tensors ----
        wq_s = wpool.tile([P, DC, F], BF16, name="wq_s")
        wk_s = wpool.tile([P, DC, F], BF16, name="wk_s")
        wv_s = wpool.tile([P, DC, F], BF16, name="wv_s")
        wo2 = wpool.tile([P, 2, D], BF16, name="wo2")
        xT = qkv.tile([P, QS, DC, 512], BF16, name="xT")
        qT = qkv.tile([P, 2, T], BF16, name="qT")        # [2 heads x dh, jb, t]
        kTz0 = qkv.tile([P, 2, T], BF16, name="kTz0")    # [k_even; 0]
        kTz1 = qkv.tile([P, 2, T], BF16, name="kTz1")    # [0; k_odd]
        vp = qkv.tile([P, TT, HG, P], BF16, name="vp")     # padded V', parity layouts
        oT = qkv.tile([P, 2, T], BF16, name="oT")        # normalized o^T [f, t]

        # ---- preamble ----
        # x slab 0 first on sync (gates the first projections)
        nc.sync.dma_start(out=xT[:, 0], in_=xt_d[:, 0:DC * 512])
        # bmask is needed by the very first S step (qs=0 is all-diagonal)
        nc.scalar.dma_start(out=bmask[:], in_=bm_d[:])
        nc.scalar.dma_start(out=wq_s[:], in_=wq_d[:])
        nc.scalar.dma_start(out=wk_s[:], in_=wk_d[:])
        # zero/ones fills arrive as DMA images (kz once; vp slab 0 now,
        # rest per slab)
        nc.sync.dma_start(out=kTz0[64:128, :, :], in_=z_d[64:128, :])
        nc.sync.dma_start(out=kTz1[0:64, :, :], in_=z_d[0:64, :])
        nc.sync.dma_start(out=vp[:, 0:4], in_=vpi_d[:, 0:4 * HG * P])

        # ---- emission helpers ----
        def _act_recip(out, in_):
            eng = nc.scalar
            inputs = [eng.lower_ap(in_)]
            for arg in (0.0, 1.0, 0.0):
                inputs.append(mybir.ImmediateValue(dtype=mybir.dt.float32, value=arg))
            return eng.add_instruction(mybir.InstActivation(
                name=nc.get_next_instruction_name(),
                func=mybir.ActivationFunctionType.Reciprocal,
                ins=inputs, outs=[eng.lower_ap(out)]))

        ei = 0

        def emit_slab(ts):
            nonlocal ei
            if ts + 1 < QS:
                nc.sync.dma_start(out=xT[:, ts + 1],
                                  in_=xt_d[:, (ts + 1) * DC * 512:(ts + 2) * DC * 512])
                t1 = ts + 1
                nc.sync.dma_start(out=vp[:, 4 * t1:4 * t1 + 4],
                                  in_=vpi_d[:, 4 * t1 * HG * P:(4 * t1 + 4) * HG * P])
            if ts == 0:
                nc.scalar.dma_start(out=wv_s[:], in_=wv_d[:])
                nc.scalar.dma_start(out=wo2[:], in_=wo_d[:])
            sl = slice(ts * 512, (ts + 1) * 512)

            def emit_proj(w_s, which, jb):
                nonlocal ei
                pp = sp_ps.tile([P, 512], F32, name="pp", tag="sp")
                for dc in range(DC):
                    nc.tensor.matmul(
                        out=pp[:],
                        lhsT=w_s[:, dc, jb * P:(jb + 1) * P],
                        rhs=xT[:, ts, dc, :],
                        start=(dc == 0),
                        stop=(dc == DC - 1),
                    )
                if which == "q":
                    nc.vector.tensor_copy(qT[:, jb, sl], pp[:])
                else:
                    nc.vector.tensor_copy(kTz0[0:64, jb, sl], pp[0:64, :])
                    nc.vector.tensor_copy(kTz1[64:128, jb, sl], pp[64:128, :])
                ei += 1

            deferred = []
            if ts == 0:
                emit_proj(wq_s, "q", 0)
                emit_proj(wk_s, "k", 0)
                emit_proj(wq_s, "q", 1)
                emit_proj(wk_s, "k", 1)
            else:
                for jb in range(2):
                    emit_proj(wq_s, "q", jb)
                for jb in range(2):
                    deferred.append(lambda b=jb: emit_proj(wk_s, "k", b))

            def emit_v(j, tt):
                nonlocal ei
                pv = sp_ps.tile([P, F], F32, name="pv", tag="sp")
                for dc in range(DC):
                    nc.tensor.matmul(
                        out=pv[:],
                        lhsT=xT[:, ts, dc, j * P:(j + 1) * P],
                        rhs=wv_s[:, dc, :],
                        start=(dc == 0),
                        stop=(dc == DC - 1),
                    )
                pvv = pv[:].rearrange("p (hp par dh) -> p hp par dh", hp=2, par=2, dh=DH)
                ve = vp[:, tt, :, :].rearrange("p (hp par) c -> p hp par c", par=2)
                nc.vector.tensor_copy(ve[:, :, 0, 0:DH], pvv[:, :, 0, :])
                nc.vector.tensor_copy(ve[:, :, 1, DH:P], pvv[:, :, 1, :])
                ei += 1
            return deferred + [(lambda a=j_, b=tt_: emit_v(a, b)) for j_, tt_ in enumerate(range(4 * ts, 4 * ts + 4))]

        state = {}

        def emit_S(qs, h, kp):
            jbh, par = h // 2, h % 2
            kTz = kTz0 if par == 0 else kTz1
            q0 = qs * 512
            spair = sp_ps.tile([P, 1024], F32, name="spair", tag="sp")
            for half in range(2):
                kt = 2 * kp + half
                k0 = kt * P
                sreg = spair[:, half * 512:(half + 1) * 512]
                lhsk = kTz[:, jbh, k0:k0 + P]
                rhsq = qT[:, jbh, :]
                if k0 >= q0:
                    d = k0 - q0
                    nc.tensor.matmul(out=sreg[:, 0:d + P], lhsT=identb[:],
                                     rhs=bmask[:, 384 - d:512], start=True, stop=False)
                    nc.tensor.matmul(out=sreg[:, d:d + P], lhsT=lhsk,
                                     rhs=rhsq[:, q0 + d:q0 + d + P],
                                     start=False, stop=(d == 384))
                    if d < 384:
                        nc.tensor.matmul(out=sreg[:, d + P:512], lhsT=lhsk,
                                         rhs=rhsq[:, q0 + d + P:q0 + 512],
                                         start=False, stop=True)
                else:
                    nc.tensor.matmul(out=sreg, lhsT=lhsk,
                                     rhs=rhsq[:, q0:q0 + 512],
                                     start=True, stop=True)
            state[(qs, h, kp)] = spair

        def emit_exp(qs, h, kp):
            spair = state[(qs, h, kp)]
            q0 = qs * 512
            d0 = max(2 * kp * P - q0, 0)
            # bias shifts the softmax uniformly (cancels in normalization);
            # keeps exp(S) well under fp8e4m3's max finite value
            pt = ptp.tile([P, 1024], BF16, name="pt")
            nc.scalar.activation(pt[:, d0:1024], spair[:, d0:1024],
                                 mybir.ActivationFunctionType.Exp,
                                 scale=0.125, bias=nbias[:])
            state[(qs, h, kp)] = (spair, pt)

        def emit_AV(qs, h, kp, npr_steps):
            _, pt = state.pop((qs, h, kp))
            q0 = qs * 512
            if kp == 0:
                state[(qs, h)] = o_ps.tile([P, 512], F32, name="opsum")
            opsum = state[(qs, h)]
            for half in range(2):
                kt = 2 * kp + half
                d = max(kt * P - q0, 0)
                nc.tensor.matmul(
                    out=opsum[:, d:512],
                    lhsT=vp[:, kt, h, :],
                    rhs=pt[:, half * 512 + d:half * 512 + 512],
                    start=(kt == 0),
                    stop=(kt == 2 * npr_steps - 1),
                )

        def emit_normA(qs, h):
            # stage the denominator row (lane-preserving copy into the
            # pair's persistent zero tile: row 64 even head, row 0 odd head)
            opsum = state[(qs, h)]
            r = DH if h % 2 == 0 else 0
            nc.vector.tensor_copy(dstages[h // 2][r:r + 1, :], opsum[r:r + 1, :])

        def emit_normE(qs, h):
            # evict o unnormalized; frees the PSUM accumulator early and
            # decouples the normalization chain from the opsum lifetime
            opsum = state.pop((qs, h))
            jbh, par = h // 2, h % 2
            rows = slice(par * 64, (par + 1) * 64)
            q0 = qs * 512
            nc.vector.tensor_copy(oT[rows, jbh, q0:q0 + 512], opsum[rows, :])

        def emit_normBa(qs, jbh):
            # PE broadcast of both heads' D + one exact DVE reciprocal for
            # the pair (cost scales with free size only)
            bcD = sp_ps.tile([P, 512], F32, name="bcD", tag="sp")
            nc.tensor.matmul(out=bcD[:], lhsT=oselP[:], rhs=dstages[jbh][:],
                             start=True, stop=True)
            rbt = stg.tile([P, 512], BF16, name="rbt")
            nc.vector.reciprocal(rbt[:], bcD[:])
            state[(qs, jbh, "rbt")] = rbt

        def emit_normF(qs, jbh):
            # in-place normalize both heads of the pair: all-SBUF bf16
            rbt = state.pop((qs, jbh, "rbt"))
            q0 = qs * 512
            sl_ = oT[:, jbh, q0:q0 + 512]
            nc.vector.tensor_mul(sl_, sl_, rbt[:])

        def emit_ytile(qs, tt, e):
            yp = sp_ps.tile([P, 1024], F32, name="yp", tag="sp")
            for jh in range(2):
                for fc in range(2):
                    nc.tensor.matmul(
                        out=yp[:, jh * 512:(jh + 1) * 512],
                        lhsT=oT[:, fc, tt * P:(tt + 1) * P],
                        rhs=wo2[:, fc, jh * 512:(jh + 1) * 512],
                        start=(fc == 0),
                        stop=(fc == 1),
                    )
            yt = ysb.tile([P, D], BF16, name="yt")
            nc.vector.tensor_copy(yt[:], yp[:])
            nc.sync.dma_start(out=y_d[tt * P:(tt + 1) * P, :], in_=yt[:])

        # ---- fused pipeline ----
        steps = []
        for qs in range(QS):
            for h in range(HG):
                nprs = 2 * qs + 2
                for kp in range(nprs):
                    steps.append((qs, h, kp, nprs))
        first_step_of_qs = {}
        step_index_of_qs = {}
        for i, (qs, h, kp, nprs) in enumerate(steps):
            if (h, kp) == (0, 0):
                first_step_of_qs[i] = qs
                step_index_of_qs[qs] = i

        # ytile deferral: qs=0 -> slab 1 (late), qs=1 -> slab 3 (early),
        # qs=2 -> slab 3 (mid), qs=3 -> drain. Keys are absolute step indices.
        ytile_keys = {}
        s1, s2, s3 = step_index_of_qs[1], step_index_of_qs[2], step_index_of_qs[3]
        for j in range(4):
            ytile_keys[(0, j)] = s1 + 8 + 2 * j
            ytile_keys[(1, j)] = s2 + 12 + 3 * j
            ytile_keys[(2, j)] = s3 + 6 + 4 * j
            ytile_keys[(3, j)] = 10 ** 9
        todo = []

        def flush(i):
            while todo and todo[0][0] <= i:
                todo.pop(0)[1]()

        def push(key, fn):
            todo.append((key, fn))
            todo.sort(key=lambda e: e[0])

        nsteps = len(steps)
        for i in range(nsteps):
            if i in first_step_of_qs:
                for vj, fn_ in enumerate(emit_slab(first_step_of_qs[i])):
                    push(i + vj, fn_)
            qs, h, kp, nprs = steps[i]
            emit_S(qs, h, kp)
            flush(i)
            if i >= 1:
                emit_exp(*steps[i - 1][:3])
            if i >= 2:
                pqs, ph_, pkp, pnprs = steps[i - 2]
                emit_AV(pqs, ph_, pkp, pnprs)
                if pkp == pnprs - 1:
                    emit_normA(pqs, ph_)
                    push(i + 2, lambda q=pqs, hh=ph_: emit_normE(q, hh))
                    if ph_ % 2 == 1:
                        ka = 3 if ph_ == 1 else 7
                        push(i + ka, lambda q=pqs, jb=ph_ // 2: emit_normBa(q, jb))
                        push(i + ka + 2, lambda q=pqs, jb=ph_ // 2: emit_normF(q, jb))
                    if ph_ == HG - 1:
                        for j, tt in enumerate(range(4 * pqs, 4 * pqs + 4)):
                            push(max(ytile_keys[(pqs, j)], i + 10 + j),
                                 lambda q=pqs, t_=tt, e=j: emit_ytile(q, t_, e))
        # drain
        emit_exp(*steps[nsteps - 1][:3])
        for i in (nsteps - 2, nsteps - 1):
            qs, h, kp, nprs = steps[i]
            emit_AV(qs, h, kp, nprs)
            if kp == nprs - 1:
                emit_normA(qs, h)
                push(10 ** 9 - 3, lambda q=qs, hh=h: emit_normE(q, hh))
                if h % 2 == 1:
                    push(10 ** 9 - 2, lambda q=qs, jb=h // 2: emit_normBa(q, jb))
                    push(10 ** 9 - 1, lambda q=qs, jb=h // 2: emit_normF(q, jb))
                if h == HG - 1:
                    for j, tt in enumerate(range(4 * qs, 4 * qs + 4)):
                        push(10 ** 9, lambda q=qs, t_=tt, e=j: emit_ytile(q, t_, e))
        for _, fn in todo:
            fn()

    nc.compile()
    return nc


def make_mask():
    # BM[k, j] = -1e9 if j < 384 + k else 0
    j = np.arange(512)[None, :]
    k = np.arange(P)[:, None]
    return np.where(j < 384 + k, np.float32(NEG), np.float32(0.0))


BF16NP = ml_dtypes.bfloat16


def make_core_inputs(x, Wq, Wk, Wv, Wo):
    bm = make_mask().astype(BF16NP)
    zeros = np.zeros((P, 4096), dtype=BF16NP)
    # initial V' image: zeros with the denominator ones-columns set
    vpi4 = np.zeros((P, TT, HG, P), dtype=np.float32)
    for h in range(HG):
        c = DH if h % 2 == 0 else 0
        vpi4[:, :, h, c] = 1.0
    vpi = np.ascontiguousarray(vpi4.reshape(P, -1)).astype(BF16NP)
    in_maps = []
    xtb = {}
    for b in range(B):
        # [p, ts, dc, j] = x[b][ts*512+j, dc*128+p]
        xt = np.ascontiguousarray(
            x[b].T.reshape(DC, P, QS, 512).transpose(1, 2, 0, 3).reshape(P, -1))
        xtb[b] = xt.astype(BF16NP)

    for c in range(NCORES):
        b, hg = c // HG, c % HG
        s = slice(hg * F, (hg + 1) * F)

        def wdr(w):  # [1024, F] -> [p, dc, f] bf16
            return np.ascontiguousarray(
                w.reshape(DC, P, F).transpose(1, 0, 2).reshape(P, -1)
            ).astype(BF16NP)

        wo = np.ascontiguousarray(
            Wo[s, :].reshape(2, P, D).transpose(1, 0, 2).reshape(P, -1)
        ).astype(BF16NP)

        in_maps.append({
            "xt": xtb[b],
            "wq": wdr(Wq[:, s]),
            "wk": wdr(Wk[:, s]),
            "wv": wdr(Wv[:, s]),
            "wo": wo,
            "bm": bm,
            "z": zeros,
            "vpi": vpi,
        })
    return in_maps


_NC_CACHE = None


def _get_nc():
    global _NC_CACHE
    if _NC_CACHE is None:
        _NC_CACHE = build_nc()
    return _NC_CACHE


def kernel(x, Wq, Wk, Wv, Wo):
    global LAST_RESULTS
    _install_ntff_hook()
    from concourse.bass_utils import run_bass_kernel_spmd

    x = np.asarray(x, dtype=np.float32)
    Wq = np.asarray(Wq, dtype=np.float32)
    Wk = np.asarray(Wk, dtype=np.float32)
    Wv = np.asarray(Wv, dtype=np.float32)
    Wo = np.asarray(Wo, dtype=np.float32)

    nc = _get_nc()
    in_maps = make_core_inputs(x, Wq, Wk, Wv, Wo)
    res = run_bass_kernel_spmd(nc, in_maps, list(range(NCORES)))
    LAST_RESULTS = res

    out = np.zeros((B, T, D), dtype=np.float32)
    for c in range(NCORES):
        out[c // HG] += res.results[c]["y"].astype(np.float32)
    return out
